# revision 13
# baseline (speedup 1.0000x reference)
"""Trainium2 Bass kernel: multi-head attention block (16 heads, d_model=1024,
B=2, Lq=Lk=2048) with mask+pair-bias softmax, out-projection, residual and
layernorm. Returns (out, attn) like the reference.

Sharding: tensor-parallel over heads for phase 1 (each of the 8 cores owns 2
heads x 2 batches: QKV projections, scores, softmax, attention output, and
P@V). Phase 2 is data-parallel over tokens (out-projection + residual + LN),
fed by a host-side reshard of the per-head attention outputs O^T.

Layout choices (all matmuls in float32r, TF32-class precision):
  - scores are computed transposed (S^T: k on partitions, q free) so that both
    score and P@V matmuls take operands in their natural layouts;
  - V is augmented with a ones column (via a padded 1025th contraction row) so
    the P@V matmul also yields the softmax denominators for free;
  - the attention-probability output is produced by PE-transposing exp(S^T+b)
    tiles; the mandatory PSUM->SBUF copy doubles as the softmax normalization
    (ACT copy with per-partition 1/sum scale).
"""

import numpy as np

N_CORES = 8
B = 2
L = 2048
DM = 1024
NH = 16
DH = 64
NT = B * L  # 4096 tokens, token t = b*L + l
EPS = 1e-6
TEMP = float(np.sqrt(DH))

_CACHE = {}


# --------------------------------------------------------------------------
# phase 1: projections + attention (per core: heads [2c, 2c+1], both batches)
# --------------------------------------------------------------------------
def _build_phase1():
    from contextlib import ExitStack

    import concourse.mybir as mybir
    import concourse.tile as tile
    from concourse import bacc

    F32 = mybir.dt.float32
    F32R = mybir.dt.float32r
    AF = mybir.ActivationFunctionType

    nc = bacc.Bacc("TRN2", target_bir_lowering=False, debug=False,
                   num_devices=N_CORES)
    qT = nc.dram_tensor("qT", [DM, NT], F32R, kind="ExternalInput").ap()
    kT = nc.dram_tensor("kT", [DM, NT], F32R, kind="ExternalInput").ap()
    vT = nc.dram_tensor("vT", [DM + 1, NT], F32R, kind="ExternalInput").ap()
    wq = nc.dram_tensor("wq", [DM, 128], F32R, kind="ExternalInput").ap()
    wk = nc.dram_tensor("wk", [DM, 128], F32R, kind="ExternalInput").ap()
    wv = nc.dram_tensor("wv", [DM + 1, 130], F32R, kind="ExternalInput").ap()
    biasT = nc.dram_tensor("biasT", [B, L, L], F32, kind="ExternalInput").ap()
    ident = nc.dram_tensor("ident", [128, 128], F32R, kind="ExternalInput").ap()
    ones_row = nc.dram_tensor("ones_row", [1, 64], F32R,
                              kind="ExternalInput").ap()
    attn_o = nc.dram_tensor("attn_o", [B, 2, L, L], F32,
                            kind="ExternalOutput").ap()
    ot_o = nc.dram_tensor("ot_o", [128, NT], F32, kind="ExternalOutput").ap()

    with tile.TileContext(nc) as tc, ExitStack() as ctx:
        const_p = ctx.enter_context(tc.tile_pool(name="const", bufs=1))
        persist = ctx.enter_context(tc.tile_pool(name="persist", bufs=1))

        ident_t = const_p.tile([128, 128], F32R)
        nc.sync.dma_start(ident_t[:], ident[:])
        ones_t = const_p.tile([1, 64], F32R)
        nc.sync.dma_start(ones_t[:], ones_row[:])

        qhT = persist.tile([128, NT], F32R)  # [head-dim pair, token]
        khT = persist.tile([128, NT], F32R)
        vh = persist.tile([128, 32 * 130], F32R)  # 32 token blocks x [h0|1|h1|1]

        # ---- projections ----
        with ExitStack() as pctx:
            wpool = pctx.enter_context(tc.tile_pool(name="wpool", bufs=1))
            pin = pctx.enter_context(tc.tile_pool(name="pin", bufs=9))
            pps = pctx.enter_context(
                tc.tile_pool(name="pps", bufs=2, space="PSUM"))

            wq_t = wpool.tile([128, 8 * 128], F32R)
            wk_t = wpool.tile([128, 8 * 128], F32R)
            wv_t = wpool.tile([128, 9 * 130], F32R)
            for kc in range(8):
                nc.sync.dma_start(wq_t[:, kc * 128:(kc + 1) * 128],
                                  wq[kc * 128:(kc + 1) * 128, :])
                nc.sync.dma_start(wk_t[:, kc * 128:(kc + 1) * 128],
                                  wk[kc * 128:(kc + 1) * 128, :])
                nc.sync.dma_start(wv_t[:, kc * 130:(kc + 1) * 130],
                                  wv[kc * 128:(kc + 1) * 128, :])
            nc.sync.dma_start(wv_t[0:1, 8 * 130:9 * 130], wv[DM:DM + 1, :])

            for tb in range(8):  # token blocks of 512
                tsl = slice(tb * 512, (tb + 1) * 512)
                qch, kch, vch = [], [], []
                for kc in range(8):
                    csl = slice(kc * 128, (kc + 1) * 128)
                    qc = pin.tile([128, 512], F32R, tag="qch")
                    nc.sync.dma_start(qc[:], qT[csl, tsl])
                    kc_t = pin.tile([128, 512], F32R, tag="kch")
                    nc.sync.dma_start(kc_t[:], kT[csl, tsl])
                    vc = pin.tile([128, 512], F32R, tag="vch")
                    nc.sync.dma_start(vc[:], vT[csl, tsl])
                    qch.append(qc)
                    kch.append(kc_t)
                    vch.append(vc)
                vlast = pin.tile([1, 512], F32R, tag="vlast")
                nc.sync.dma_start(vlast[:], vT[DM:DM + 1, tsl])

                ps_q = pps.tile([128, 512], F32, tag="psq")
                ps_k = pps.tile([128, 512], F32, tag="psk")
                for kc in range(8):
                    wsl = slice(kc * 128, (kc + 1) * 128)
                    nc.tensor.matmul(ps_q[:], wq_t[:, wsl], qch[kc][:],
                                     start=(kc == 0), stop=(kc == 7))
                for kc in range(8):
                    wsl = slice(kc * 128, (kc + 1) * 128)
                    nc.tensor.matmul(ps_k[:], wk_t[:, wsl], kch[kc][:],
                                     start=(kc == 0), stop=(kc == 7))
                nc.vector.tensor_copy(qhT[:, tsl], ps_q[:])
                nc.vector.tensor_copy(khT[:, tsl], ps_k[:])

                for t2 in range(4):  # vh: token sub-blocks of 128
                    blk = tb * 4 + t2
                    ssl = slice(t2 * 128, (t2 + 1) * 128)
                    ps_v = pps.tile([128, 130], F32, tag="psv")
                    for kc in range(8):
                        nc.tensor.matmul(
                            ps_v[:], vch[kc][:, ssl],
                            wv_t[:, kc * 130:(kc + 1) * 130],
                            start=(kc == 0), stop=False)
                    nc.tensor.matmul(ps_v[:], vlast[:, ssl],
                                     wv_t[0:1, 8 * 130:9 * 130],
                                     start=False, stop=True)
                    nc.vector.tensor_copy(
                        vh[:, blk * 130:(blk + 1) * 130], ps_v[:])

        # ---- attention ----
        with ExitStack() as actx:
            biasp = actx.enter_context(tc.tile_pool(name="biasp", bufs=18))
            stps = actx.enter_context(
                tc.tile_pool(name="stps", bufs=2, space="PSUM"))
            ops_p = actx.enter_context(
                tc.tile_pool(name="ops", bufs=1, space="PSUM"))
            atps = actx.enter_context(
                tc.tile_pool(name="atps", bufs=1, space="PSUM"))
            smps = actx.enter_context(
                tc.tile_pool(name="smps", bufs=1, space="PSUM"))
            stbp = actx.enter_context(tc.tile_pool(name="stbp", bufs=3))
            ptp = actx.enter_context(tc.tile_pool(name="ptp", bufs=10))
            atsb = actx.enter_context(tc.tile_pool(name="atsb", bufs=3))
            smsb = actx.enter_context(tc.tile_pool(name="smsb", bufs=6))

            for b in range(2):
                for qb in range(4):  # q blocks of 512
                    q_tok = slice(b * L + qb * 512, b * L + (qb + 1) * 512)
                    q_l = slice(qb * 512, (qb + 1) * 512)
                    btiles = []
                    for kb in range(16):
                        bt = biasp.tile([128, 512], F32, tag="bias")
                        nc.sync.dma_start(
                            bt[:], biasT[b, kb * 128:(kb + 1) * 128, q_l])
                        btiles.append(bt)
                    for h in range(2):
                        hsl = slice(h * 64, (h + 1) * 64)
                        o_ps = ops_p.tile([65, 512], F32, tag="o")
                        pts = []
                        for kb2 in range(8):  # pairs of k-blocks
                            st_ps = stps.tile([128, 1024], F32, tag="st")
                            stb = stbp.tile([128, 1024], F32, tag="stb")
                            ptt = ptp.tile([128, 1024], F32R, tag="pt")
                            for j in range(2):
                                kb = kb2 * 2 + j
                                k_tok = slice(b * L + kb * 128,
                                              b * L + (kb + 1) * 128)
                                jsl = slice(j * 512, (j + 1) * 512)
                                nc.tensor.matmul(
                                    st_ps[:, jsl], khT[hsl, k_tok],
                                    qhT[hsl, q_tok], start=True, stop=True)
                                nc.vector.tensor_add(
                                    stb[:, jsl], st_ps[:, jsl], btiles[kb][:])
                            nc.scalar.activation(ptt[:], stb[:], AF.Exp)
                            for j in range(2):
                                kb = kb2 * 2 + j
                                blk = b * 16 + kb
                                voff = blk * 130 + h * 65
                                jsl = slice(j * 512, (j + 1) * 512)
                                nc.tensor.matmul(
                                    o_ps[:], vh[:, voff:voff + 65],
                                    ptt[:, jsl],
                                    start=(kb == 0), stop=(kb == 15))
                            pts.append(ptt)

                        # softmax stats: sums sit in o_ps row 64
                        sums_t = smsb.tile([65, 512], F32, tag="sums")
                        nc.vector.tensor_copy(sums_t[64:65, :], o_ps[64:65, :])
                        recip_row = smsb.tile([1, 512], F32R, tag="rrow")
                        recTs = []
                        for qs in range(4):
                            qss = slice(qs * 128, (qs + 1) * 128)
                            sT_ps = smps.tile([128, 1], F32, tag="sm")
                            nc.tensor.transpose(
                                sT_ps[:], sums_t[64:65, qss],
                                ident_t[64:65, 64:65].bitcast(F32))
                            recT = smsb.tile([128, 1], F32, tag="recT")
                            nc.vector.reciprocal(recT[:], sT_ps[:])
                            recTs.append(recT)
                            rrow_ps = smps.tile([1, 128], F32, tag="sm")
                            nc.tensor.transpose(
                                rrow_ps[:], recT[:], ident_t[:].bitcast(F32))
                            nc.vector.tensor_copy(recip_row[:, qss],
                                                  rrow_ps[:])

                        # normalize O^T rows for this head, write out
                        bc_ps = smps.tile([64, 512], F32, tag="sm")
                        nc.tensor.matmul(bc_ps[:], ones_t[:], recip_row[:],
                                         start=True, stop=True)
                        o_sb = smsb.tile([64, 512], F32, tag="osb")
                        nc.vector.tensor_copy(o_sb[:], o_ps[0:64, :])
                        on_sb = smsb.tile([64, 512], F32, tag="on")
                        nc.vector.tensor_mul(on_sb[:], o_sb[:], bc_ps[:])
                        nc.sync.dma_start(ot_o[hsl, q_tok], on_sb[:])

                        # attention probabilities: transpose + scaled copy
                        for qs in range(4):
                            qss = slice(qs * 128, (qs + 1) * 128)
                            for half in range(2):
                                at_ps = atps.tile([128, 1024], F32R, tag="at")
                                for kk in range(8):
                                    kb = half * 8 + kk
                                    ptt = pts[kb // 2]
                                    joff = (kb % 2) * 512
                                    nc.tensor.transpose(
                                        at_ps[:, kk * 128:(kk + 1) * 128],
                                        ptt[:, joff + qs * 128:
                                            joff + (qs + 1) * 128],
                                        ident_t[:])
                                at_sb = atsb.tile([128, 1024], F32, tag="atsb")
                                nc.scalar.activation(
                                    at_sb[:], at_ps[:].bitcast(F32), AF.Copy,
                                    scale=recTs[qs][:])
                                nc.sync.dma_start(
                                    attn_o[b, h,
                                           qb * 512 + qs * 128:
                                           qb * 512 + (qs + 1) * 128,
                                           half * 1024:(half + 1) * 1024],
                                    at_sb[:])
    nc.compile()
    return nc


# --------------------------------------------------------------------------
# phase 2: out-projection + residual + layernorm (per core: 512 token rows)
# --------------------------------------------------------------------------
def _build_phase2():
    from contextlib import ExitStack

    import concourse.mybir as mybir
    import concourse.tile as tile
    from concourse import bacc

    F32 = mybir.dt.float32
    F32R = mybir.dt.float32r
    AF = mybir.ActivationFunctionType

    nc = bacc.Bacc("TRN2", target_bir_lowering=False, debug=False,
                   num_devices=N_CORES)
    otg = nc.dram_tensor("otg", [DM, 512], F32R, kind="ExternalInput").ap()
    woT = nc.dram_tensor("woT", [DM, DM], F32R, kind="ExternalInput").ap()
    resid = nc.dram_tensor("resid", [512, DM], F32, kind="ExternalInput").ap()
    gammab = nc.dram_tensor("gammab", [128, DM], F32,
                            kind="ExternalInput").ap()
    betab = nc.dram_tensor("betab", [128, DM], F32, kind="ExternalInput").ap()
    y_o = nc.dram_tensor("y_o", [512, DM], F32, kind="ExternalOutput").ap()

    with tile.TileContext(nc) as tc, ExitStack() as ctx:
        wpool = ctx.enter_context(tc.tile_pool(name="wpool", bufs=1))
        sb = ctx.enter_context(tc.tile_pool(name="sb", bufs=3))
        small = ctx.enter_context(tc.tile_pool(name="small", bufs=3))
        pps = ctx.enter_context(tc.tile_pool(name="pps", bufs=2, space="PSUM"))

        ot_t = wpool.tile([128, 8 * 512], F32R)
        wo_t = wpool.tile([128, 8 * 1024], F32R)
        gb_t = wpool.tile([128, DM], F32)
        bb_t = wpool.tile([128, DM], F32)
        eps_t = wpool.tile([128, 1], F32)
        nc.vector.memset(eps_t[:], float(EPS))
        for kc in range(8):
            csl = slice(kc * 128, (kc + 1) * 128)
            nc.sync.dma_start(ot_t[:, kc * 512:(kc + 1) * 512], otg[csl, :])
            nc.sync.dma_start(wo_t[:, kc * 1024:(kc + 1) * 1024], woT[csl, :])
        nc.sync.dma_start(gb_t[:], gammab[:])
        nc.sync.dma_start(bb_t[:], betab[:])

        for tb in range(4):  # 128 token rows each
            tsl = slice(tb * 128, (tb + 1) * 128)
            y_ps = pps.tile([128, 1024], F32, tag="y")
            for jh in range(2):
                jsl = slice(jh * 512, (jh + 1) * 512)
                for kc in range(8):
                    lh = ot_t[:, kc * 512 + tb * 128:kc * 512 + (tb + 1) * 128]
                    rh = wo_t[:, kc * 1024 + jh * 512:kc * 1024 + (jh + 1) * 512]
                    nc.tensor.matmul(y_ps[:, jsl], lh, rh,
                                     start=(kc == 0), stop=(kc == 7))
            rs = sb.tile([128, DM], F32, tag="rs")
            nc.sync.dma_start(rs[:], resid[tsl, :])
            y_sb = sb.tile([128, DM], F32, tag="ysb")
            nc.vector.tensor_add(y_sb[:], y_ps[:], rs[:])

            ssum = small.tile([128, 1], F32, tag="ssum")
            nc.vector.reduce_sum(ssum[:], y_sb[:],
                                 axis=mybir.AxisListType.X)
            mu = small.tile([128, 1], F32, tag="mu")
            nc.scalar.mul(mu[:], ssum[:], 1.0 / DM)
            xc = sb.tile([128, DM], F32, tag="xc")
            nc.vector.tensor_scalar_sub(xc[:], y_sb[:], mu[:])
            sq = sb.tile([128, DM], F32, tag="sq")
            var_s = small.tile([128, 1], F32, tag="vars")
            nc.scalar.activation(sq[:], xc[:], AF.Square, accum_out=var_s[:])
            sd = small.tile([128, 1], F32, tag="sd")
            nc.scalar.activation(sd[:], var_s[:], AF.Sqrt, bias=eps_t[:],
                                 scale=1.0 / DM)
            rinv = small.tile([128, 1], F32, tag="rinv")
            nc.vector.reciprocal(rinv[:], sd[:])
            yn = sb.tile([128, DM], F32, tag="yn")
            nc.vector.tensor_scalar_mul(yn[:], xc[:], rinv[:])
            yg = sb.tile([128, DM], F32, tag="yg")
            nc.vector.tensor_mul(yg[:], yn[:], gb_t[:])
            yf = sb.tile([128, DM], F32, tag="yf")
            nc.vector.tensor_add(yf[:], yg[:], bb_t[:])
            nc.sync.dma_start(y_o[tsl, :], yf[:])
    nc.compile()
    return nc


def _get_phase(name):
    if name not in _CACHE:
        if name == "p1":
            _CACHE[name] = _build_phase1()
        else:
            _CACHE[name] = _build_phase2()
    return _CACHE[name]


class _Runner:
    """Cached PJRT runner for one compiled Bass module on 8 cores.

    Mirrors concourse.bass2jax.run_bass_via_pjrt's multi-core path, but keeps
    the jitted executable and device-resident inputs so repeated calls measure
    device execution rather than re-trace/transfer."""

    def __init__(self, nc):
        import jax
        import jax.numpy as jnp
        import concourse.mybir as mybir
        from concourse import bass2jax
        from jax.experimental.shard_map import shard_map
        from jax.sharding import Mesh, NamedSharding, PartitionSpec

        bass2jax.install_neuronx_cc_hook()
        self.nc = nc
        partition_name = (nc.partition_id_tensor.name
                          if nc.partition_id_tensor else None)
        in_names, out_names, out_avals = [], [], []
        for alloc in nc.m.functions[0].allocations:
            if not isinstance(alloc, mybir.MemoryLocationSet):
                continue
            name = alloc.memorylocations[0].name
            if alloc.kind == "ExternalInput":
                if name != partition_name:
                    in_names.append(name)
            elif alloc.kind == "ExternalOutput":
                out_names.append(name)
                out_avals.append(jax.core.ShapedArray(
                    tuple(alloc.tensor_shape), mybir.dt.np(alloc.dtype)))
        n_params = len(in_names)
        n_outs = len(out_names)
        all_in_names = list(in_names) + list(out_names)
        if partition_name is not None:
            all_in_names.append(partition_name)
        self.in_names = in_names
        self.out_names = out_names
        self.out_avals = out_avals

        def _body(*args):
            operands = list(args)
            if partition_name is not None:
                operands.append(bass2jax.partition_id_tensor())
            outs = bass2jax._bass_exec_p.bind(
                *operands,
                out_avals=tuple(out_avals),
                in_names=tuple(all_in_names),
                out_names=tuple(out_names),
                lowering_input_output_aliases=(),
                sim_require_finite=True,
                sim_require_nnan=True,
                nc=nc,
            )
            return tuple(outs)

        devices = jax.devices()[:N_CORES]
        self.mesh = Mesh(np.asarray(devices), ("core",))
        spec = PartitionSpec("core")
        self.sharding = NamedSharding(self.mesh, spec)
        in_specs = (spec,) * (n_params + n_outs)
        out_specs = (spec,) * n_outs
        donate = tuple(range(n_params, n_params + n_outs))
        self.sharded = jax.jit(
            shard_map(_body, mesh=self.mesh, in_specs=in_specs,
                      out_specs=out_specs, check_rep=False),
            donate_argnums=donate, keep_unused=True)

        zero_shardings = tuple(self.sharding for _ in out_avals)

        def _zeros():
            return tuple(
                jnp.zeros((N_CORES * a.shape[0], *a.shape[1:]), a.dtype)
                for a in out_avals)

        self.make_zeros = jax.jit(_zeros, out_shardings=zero_shardings)
        self.in_dev = None

    def put_inputs(self, in_maps):
        import jax
        concat = [
            np.concatenate([np.asarray(m[n]) for m in in_maps], axis=0)
            for n in self.in_names
        ]
        self.in_dev = [jax.device_put(a, self.sharding) for a in concat]

    def run(self):
        import jax
        import time as _time
        zeros = self.make_zeros()
        jax.block_until_ready(zeros)
        t0 = _time.perf_counter()
        outs = self.sharded(*self.in_dev, *zeros)
        jax.block_until_ready(outs)
        self.last_dt = _time.perf_counter() - t0
        return outs

    def run_to_numpy(self):
        outs = self.run()
        res = []
        for c in range(N_CORES):
            d = {}
            for i, name in enumerate(self.out_names):
                a = self.out_avals[i]
                d[name] = np.asarray(outs[i]).reshape(
                    N_CORES, *a.shape)[c]
            res.append(d)
        return res


def _get_runner(name):
    key = name + "_runner"
    if key not in _CACHE:
        _CACHE[key] = _Runner(_get_phase(name))
    return _CACHE[key]


# --------------------------------------------------------------------------
# host driver
# --------------------------------------------------------------------------
LAST_EXEC_NS = None
LAST_PHASE_SECONDS = (None, None)


def kernel(q, k, v, mask, pair_bias, Wq, Wk, Wv, Wo, gamma, beta):
    global LAST_EXEC_NS, LAST_PHASE_SECONDS
    f32 = np.float32
    q = np.asarray(q, f32)
    k = np.asarray(k, f32)
    v = np.asarray(v, f32)
    mask = np.asarray(mask)
    pair_bias = np.asarray(pair_bias, f32)
    Wq = np.asarray(Wq, f32)
    Wk = np.asarray(Wk, f32)
    Wv = np.asarray(Wv, f32)
    Wo = np.asarray(Wo, f32)
    gamma = np.asarray(gamma, f32)
    beta = np.asarray(beta, f32)

    qf = q.reshape(NT, DM)
    qT = np.ascontiguousarray(qf.T)
    kT = np.ascontiguousarray(k.reshape(NT, DM).T)
    vT = np.empty((DM + 1, NT), f32)
    vT[:DM] = v.reshape(NT, DM).T
    vT[DM] = 1.0

    bias = pair_bias[:, 0] + np.where(mask == 0, f32(-1e9), f32(0.0))
    biasT = np.ascontiguousarray(bias.transpose(0, 2, 1))

    eye = np.eye(128, dtype=f32)
    ones_row = np.ones((1, 64), f32)

    in_maps = []
    for c in range(N_CORES):
        rsl = slice(c * 128, (c + 1) * 128)
        wq_c = np.ascontiguousarray(Wq[rsl, :].T) / f32(TEMP)
        wk_c = np.ascontiguousarray(Wk[rsl, :].T)
        wv_base = Wv[rsl, :].T  # [DM, 128]
        wv_c = np.zeros((DM + 1, 130), f32)
        wv_c[:DM, 0:64] = wv_base[:, 0:64]
        wv_c[:DM, 65:129] = wv_base[:, 64:128]
        wv_c[DM, 64] = 1.0
        wv_c[DM, 129] = 1.0
        in_maps.append(dict(qT=qT, kT=kT, vT=vT, wq=wq_c, wk=wk_c, wv=wv_c,
                            biasT=biasT, ident=eye, ones_row=ones_row))

    r1 = _get_runner("p1")
    r1.put_inputs(in_maps)
    res1 = r1.run_to_numpy()

    attn = np.empty((B, NH, L, L), f32)
    OT = np.empty((DM, NT), f32)
    for c in range(N_CORES):
        attn[:, 2 * c:2 * c + 2] = res1[c]["attn_o"]
        OT[c * 128:(c + 1) * 128] = res1[c]["ot_o"]

    woT = np.ascontiguousarray(Wo.T)
    gammab = np.broadcast_to(gamma, (128, DM)).copy()
    betab = np.broadcast_to(beta, (128, DM)).copy()
    in_maps2 = []
    for c in range(N_CORES):
        tsl = slice(c * 512, (c + 1) * 512)
        in_maps2.append(dict(
            otg=np.ascontiguousarray(OT[:, tsl]), woT=woT,
            resid=np.ascontiguousarray(qf[tsl, :]),
            gammab=gammab, betab=betab))

    r2 = _get_runner("p2")
    r2.put_inputs(in_maps2)
    res2 = r2.run_to_numpy()

    out = np.empty((NT, DM), f32)
    for c in range(N_CORES):
        out[c * 512:(c + 1) * 512] = res2[c]["y_o"]

    LAST_PHASE_SECONDS = (r1.last_dt, r2.last_dt)
    LAST_EXEC_NS = int((r1.last_dt + r2.last_dt) * 1e9)
    return out.reshape(B, L, DM), attn
